# revision 17
# baseline (speedup 1.0000x reference)
"""Trainium2 Bass kernel for AttentionWithCAE.

Reference computation (B=8, N=1024, C=768, H=12, hd=64):
    qkv  = x @ qkv_w.T + concat(q_bias, 0, v_bias)
    q,k,v per head; attn = softmax(mask(q*scale @ k.T)); out = attn @ v
    final = out @ proj_w.T + proj_b

Sharding: pure data parallel -- batch b on core b, weights replicated,
no collectives.

Key optimization vs the v1 kernel: masked keys (~50% of the 1024) are
removed on the HOST (exact algebra: their exp() is 0, they contribute
nothing). Each batch's unmasked keys are gathered into a compact
[C, NK] tensor (NK = KT*128, KT = ceil(max_b n_unmasked / 128), 5 for
the reference data), cutting the scores matmuls, exp() activations,
attn@v matmuls and the k/v projections by ~40%.

Device-side layout (per core, all zero-transpose):
  - xT [C, N] bf16 full tokens (q path); xk [C, NK] bf16 gathered keys
    (k/v path); wq (pre-scaled by SCALE) / wk / wv [C, 768] bf16;
    pwT [C, C] bf16.
  - qT[p] [128, N]: q features of heads 2p (rows 0:64) / 2p+1 (64:128);
    kT[p] [128, NK] likewise: exactly the lhsT/rhs layout the scores
    matmul needs (contraction over head_dim; two concurrent K=64
    matmuls on PE row halves).
  - scores computed transposed [k, q]; the key-dependent pad-mask bias
    is a per-partition bias folded into the Exp activation.
  - v emitted token-major into v65 tiles [128, 12*65]: per head 64
    v-columns plus a baked ones column, so each attn@v matmul also
    yields the softmax denominators (PSUM row 64).
  - denominators -> reciprocal (direct from PSUM row) -> DRAM-bounce
    partition-broadcast -> normalize into aoT [C-major, N] which feeds
    proj; v_bias folds into an effective proj bias on the host.

Schedule: the steady state is paced by the Exp ACTs (2 per (pair, kt)
on ScalarE). Per kt iteration the PE runs 4 scores matmuls (row-packed
pairs) + 4 attn@v matmuls of the previous pair; the next pair's q/k
projection tiles are emitted inside the kt loop (kt==2/kt==4) where the
ScalarE backlog hides them. PSUM: 4 banks scores (double-buffered
[128,1024], rotation shared with q/k-proj and final proj) + 4 banks AV.
"""

import sys

sys.path.insert(0, "/opt/trn_rl_repo")

from contextlib import ExitStack

import numpy as np
import ml_dtypes

import concourse.bass as bass
import concourse.bacc as bacc
import concourse.mybir as mybir
from concourse import tile
from concourse.bass_utils import run_bass_kernel_spmd

B, N, C = 8, 1024, 768
H, HD = 12, 64
F3 = 3 * C
SCALE = HD ** -0.5
F32 = mybir.dt.float32
BF16 = mybir.dt.bfloat16
Act = mybir.ActivationFunctionType

MASK_NEG = -30000.0

CT = C // 128  # 6 contraction tiles
NPAIR = H // 2  # 6 head pairs

_CACHE = {}


def _build_nc(KT):
    NK = KT * 128
    nc = bacc.Bacc(None, target_bir_lowering=False)

    # All matrix inputs arrive in a host-transposed "mega-tile" layout
    # [128, CT*cols]: partition p, column c*cols+j holds element
    # [c*128+p, j] of the logical [C, cols] matrix. This makes every DMA
    # per-partition contiguous (1.5-4KB packets instead of the
    # sub-2KB strided lines that made the v1 input load DMA-overhead
    # bound). wq/wk/pw additionally group columns by use order (see
    # kernel()) so pair-0 weight chunks can load first.
    xT_d = nc.declare_dram_parameter("xT", [128, CT * N], BF16, isOutput=False)
    xk_d = nc.declare_dram_parameter("xk", [128, CT * NK], BF16, isOutput=False)
    wq_d = nc.declare_dram_parameter("wq", [128, CT * C], BF16, isOutput=False)
    wk_d = nc.declare_dram_parameter("wk", [128, CT * C], BF16, isOutput=False)
    wv_d = nc.declare_dram_parameter("wv", [128, CT * C], BF16, isOutput=False)
    pw_d = nc.declare_dram_parameter("pwT", [128, CT * C], BF16, isOutput=False)
    # biases arrive host-transposed [128, tiles] so the DMA is contiguous
    qkb_d = nc.declare_dram_parameter("qkb", [128, CT], F32, isOutput=False)
    mb_d = nc.declare_dram_parameter("mb", [128, KT], F32, isOutput=False)
    pb_d = nc.declare_dram_parameter("pb", [128, CT], F32, isOutput=False)
    out_d = nc.declare_dram_parameter("out", [C, N], F32, isOutput=True)

    r_d = nc.dram_tensor("r_scratch", [H, N], F32)

    with ExitStack() as ctx:
        tc = ctx.enter_context(tile.TileContext(nc))
        pool = ctx.enter_context(tc.tile_pool(name="main", bufs=1))
        psum = ctx.enter_context(tc.tile_pool(name="psum", bufs=1, space="PSUM"))

        qb_sb = pool.tile([128, CT], F32)
        mb_sb = pool.tile([128, KT], F32)
        pb_sb = pool.tile([128, CT], F32)

        # Mega-tiles in SBUF. Slice helpers below recover the per-c-tile
        # views the matmuls need.
        wq_sb = pool.tile([128, CT * C], BF16, name="wq_sb")
        wk_sb = pool.tile([128, CT * C], BF16, name="wk_sb")
        wv_sb = pool.tile([128, CT * C], BF16, name="wv_sb")
        pw_sb = pool.tile([128, CT * C], BF16, name="pw_sb")
        xT_sb = pool.tile([128, CT * N], BF16, name="xT_sb")
        xk_sb = pool.tile([128, CT * NK], BF16, name="xk_sb")

        def chunk_dma(sb, dr, n_chunks):
            w = sb.shape[1]
            step = w // n_chunks
            for i in range(n_chunks):
                nc.sync.dma_start(
                    out=sb[:, i * step : (i + 1) * step],
                    in_=dr[:, i * step : (i + 1) * step],
                )

        # Load order = first use: pair-0 q/k weight chunks + xT + xk first,
        # then the remaining pairs' weight chunks, v weights, proj weights.
        # wq/wk layout groups 768 columns per pair (6 c-slices of 128).
        nc.sync.dma_start(out=wq_sb[:, 0:C], in_=wq_d[:, 0:C])
        chunk_dma(xT_sb, xT_d, 3)
        nc.sync.dma_start(out=wk_sb[:, 0:C], in_=wk_d[:, 0:C])
        nc.sync.dma_start(out=qb_sb, in_=qkb_d[:, :])
        chunk_dma(xk_sb, xk_d, 2)
        nc.sync.dma_start(out=mb_sb, in_=mb_d[:, :])
        chunk_dma(wv_sb, wv_d, 2)
        for p in range(1, NPAIR):
            nc.sync.dma_start(
                out=wq_sb[:, p * C : (p + 1) * C], in_=wq_d[:, p * C : (p + 1) * C]
            )
            nc.sync.dma_start(
                out=wk_sb[:, p * C : (p + 1) * C], in_=wk_d[:, p * C : (p + 1) * C]
            )
        chunk_dma(pw_sb, pw_d, 2)
        nc.sync.dma_start(out=pb_sb, in_=pb_d[:, :])

        # wq/wk/pw column j of logical tile (group, c) = group*C + c*128 + j
        def wslice(sb, group, c):
            return sb[:, group * C + c * 128 : group * C + (c + 1) * 128]

        # xT/xk column t of c-tile c = c*width + t
        def xslice(sb, width, c, lo, hi):
            return sb[:, c * width + lo : c * width + hi]

        qT = [
            pool.tile([128, N], BF16, tag="qT", bufs=NPAIR, name=f"qT{p}")
            for p in range(NPAIR)
        ]
        kT = [
            pool.tile([128, NK], BF16, tag="kT", bufs=NPAIR, name=f"kT{p}")
            for p in range(NPAIR)
        ]
        v65 = [
            pool.tile([128, H * 65], BF16, tag="v65", bufs=KT, name=f"v65_{i}")
            for i in range(KT)
        ]
        aoT = [
            pool.tile([128, N], BF16, tag="aoT", bufs=CT, name=f"aoT{i}")
            for i in range(CT)
        ]

        qps = {}

        def emit_q_half(p, half):
            # q-projection emitted in two 3-c-tile halves so no single PE
            # burst outruns the ScalarE exp backlog
            if half == 0:
                qps[p] = psum.tile([128, N], F32, tag="psS", bufs=2, name=f"ps_q{p}")
            ps = qps[p]
            for c in range(3 * half, 3 * half + 3):
                for qc in range(2):
                    nc.tensor.matmul(
                        ps[:, qc * 512 : (qc + 1) * 512],
                        lhsT=wslice(wq_sb, p, c),
                        rhs=xslice(xT_sb, N, c, qc * 512, (qc + 1) * 512),
                        start=(c == 0),
                        stop=(c == CT - 1),
                    )
            if half == 1:
                nc.scalar.activation(
                    qT[p], ps, Act.Identity, bias=qb_sb[:, p : p + 1]
                )

        kps = {}

        def emit_k_half(p, half):
            if half == 0:
                kps[p] = psum.tile([128, N], F32, tag="psS", bufs=2, name=f"ps_k{p}")
            ps = kps[p]
            chunks = [(0, min(NK, 512))]
            if NK > 512:
                chunks.append((512, NK))
            for c in range(3 * half, 3 * half + 3):
                for lo, hi in chunks:
                    nc.tensor.matmul(
                        ps[:, lo:hi],
                        lhsT=wslice(wk_sb, p, c),
                        rhs=xslice(xk_sb, NK, c, lo, hi),
                        start=(c == 0),
                        stop=(c == CT - 1),
                    )
            if half == 1:
                nc.scalar.activation(kT[p], ps[:, 0:NK], Act.Copy)

        def emit_q_tile(p):
            emit_q_half(p, 0)
            emit_q_half(p, 1)

        def emit_k_tile(p):
            emit_k_half(p, 0)
            emit_k_half(p, 1)

        def emit_v_tile(kt):
            vps = psum.tile(
                [128, 768], F32, tag=f"pv{kt % 2}", bufs=1, name=f"ps_v{kt}"
            )
            for c in range(CT):
                nc.tensor.matmul(
                    vps[:, 0:512],
                    lhsT=xslice(xk_sb, NK, c, kt * 128, (kt + 1) * 128),
                    rhs=xslice(wv_sb, C, c, 0, 512),
                    start=(c == 0),
                    stop=(c == CT - 1),
                )
                nc.tensor.matmul(
                    vps[:, 512:768],
                    lhsT=xslice(xk_sb, NK, c, kt * 128, (kt + 1) * 128),
                    rhs=xslice(wv_sb, C, c, 512, 768),
                    start=(c == 0),
                    stop=(c == CT - 1),
                )
            v3 = v65[kt].rearrange("p (h j) -> p h j", j=65)
            nc.vector.tensor_copy(
                out=v3[:, :, 0:64], in_=vps.rearrange("p (h j) -> p h j", j=64)
            )
            nc.vector.memset(v3[:, :, 64:65], 1.0)

        def emit_av_kt(p, kt, atiles, pav):
            for hi in range(2):
                h = 2 * p + hi
                for qc in range(2):
                    nc.tensor.matmul(
                        pav[hi][0:65, qc * 512 : (qc + 1) * 512],
                        lhsT=v65[kt][:, h * 65 : (h + 1) * 65],
                        rhs=atiles[kt][
                            :, hi * 1024 + qc * 512 : hi * 1024 + (qc + 1) * 512
                        ],
                        start=(kt == 0),
                        stop=(kt == KT - 1),
                    )

        def finish_pair(p, pav):
            # Fast [65, 512] copies evict the unnormalized AV plus the
            # denominator row (PSUM row 64) to SBUF right away, releasing the
            # PSUM banks for the next pair's AV. The slow normalization chain
            # (reciprocal -> DRAM-bounce broadcast -> mul) then runs entirely
            # from SBUF. NOTE: reciprocal_approx_fast directly on a PSUM
            # source returns garbage on HW (sim accepts it).
            for hi in range(2):
                h = 2 * p + hi
                un = pool.tile([64, N], F32, tag=f"un{hi}", bufs=2, name=f"un{h}")
                srow = pool.tile([1, N], F32, tag="srow", bufs=2, name=f"s{h}")
                nc.vector.tensor_copy(out=un, in_=pav[hi][0:64, :])
                # NOTE: reciprocal_approx_fast needs a partition-0 SBUF
                # source (PSUM or partition-offset sources return garbage
                # on HW; sim accepts both) — hence the srow bounce.
                nc.vector.tensor_copy(out=srow, in_=pav[hi][64:65, :])
                r_row = pool.tile([1, N], F32, tag="rrow", bufs=2, name=f"r{h}")
                nc.vector.reciprocal_approx_fast(out=r_row, in_=srow)
                nc.sync.dma_start(out=r_d[h : h + 1, :], in_=r_row)
                r2 = pool.tile([64, N], F32, tag="r2", bufs=4, name=f"r2_{h}")
                nc.sync.dma_start(out=r2, in_=r_d[h : h + 1, :].to_broadcast([64, N]))
                # normalization muls split across the otherwise-idle GPSIMD
                # (all-SBUF operands) and DVE so the two head-chains overlap
                eng = nc.gpsimd if hi == 0 else nc.vector
                eng.tensor_mul(
                    out=aoT[p][hi * 64 : (hi + 1) * 64, :],
                    in0=un,
                    in1=r2,
                )

        # proj pass 1: c-tiles 0..3 (pairs 0..3, ready once finish_pair(3)
        # ran) accumulated into bf16 partials with the output bias; pass 2
        # adds c-tiles 4..5 after the last pair finishes. Pass 1 is emitted
        # inside pair 5's kt loop where the PE has slack under the exp pace.
        osb1 = [
            pool.tile([128, N], BF16, tag="osb1", bufs=CT, name=f"op1_{i}")
            for i in range(CT)
        ]

        def emit_proj_pass1(ot):
            ps = psum.tile([128, N], F32, tag="psS", bufs=2, name=f"ps_p1_{ot}")
            for c in range(4):
                for qc in range(2):
                    nc.tensor.matmul(
                        ps[:, qc * 512 : (qc + 1) * 512],
                        lhsT=wslice(pw_sb, ot, c),
                        rhs=aoT[c][:, qc * 512 : (qc + 1) * 512],
                        start=(c == 0),
                        stop=(c == 3),
                    )
            nc.scalar.activation(
                osb1[ot], ps, Act.Identity, bias=pb_sb[:, ot : ot + 1]
            )

        emit_q_tile(0)
        emit_k_tile(0)
        pass1_done = set()
        pav_prev = None
        atn_prev = None
        for p in range(NPAIR):
            if p > 0:
                pav_prev = [
                    psum.tile(
                        [128, 1024], F32, tag=f"pv{hi}", bufs=1,
                        name=f"pav{2 * (p - 1) + hi}",
                    )
                    for hi in range(2)
                ]
            atiles = []
            for kt in range(KT):
                # PE filler work first (so later scores/exps never wait
                # behind it via psum-slot WAR in the same engine stream)
                if p < NPAIR - 1:
                    # self-contained inserts (alloc+matmuls+evict) only: a
                    # psum tile held across kt iterations would deadlock the
                    # 2-deep psS rotation
                    nxt = p + 1
                    if kt == min(1, KT - 1):
                        emit_q_tile(nxt)
                    if kt == min(3, KT - 1):
                        emit_k_tile(nxt)
                else:
                    if kt < 5 and kt < CT:
                        emit_proj_pass1(kt)
                        pass1_done.add(kt)
                ps0 = psum.tile(
                    [128, N], F32, tag="psS", bufs=2, name=f"ps_s{2 * p}_{kt}"
                )
                ps1 = psum.tile(
                    [128, N], F32, tag="psS", bufs=2, name=f"ps_s{2 * p + 1}_{kt}"
                )
                for qc in range(2):
                    # row-packed pair: even head rows 0-63, odd head 64-127
                    nc.tensor.matmul(
                        ps0[:, qc * 512 : (qc + 1) * 512],
                        lhsT=kT[p][0:64, kt * 128 : (kt + 1) * 128],
                        rhs=qT[p][0:64, qc * 512 : (qc + 1) * 512],
                        start=True,
                        stop=True,
                    )
                    nc.tensor.matmul(
                        ps1[:, qc * 512 : (qc + 1) * 512],
                        lhsT=kT[p][64:128, kt * 128 : (kt + 1) * 128],
                        rhs=qT[p][64:128, qc * 512 : (qc + 1) * 512],
                        start=True,
                        stop=True,
                    )
                if p > 0:
                    emit_av_kt(p - 1, kt, atn_prev, pav_prev)
                if p == 0:
                    emit_v_tile(kt)
                at = pool.tile(
                    [128, 2048], BF16, tag="attn", bufs=2 * KT, name=f"at{p}_{kt}"
                )
                nc.scalar.activation(
                    at[:, 0:1024], ps0, Act.Exp, bias=mb_sb[:, kt : kt + 1]
                )
                nc.scalar.activation(
                    at[:, 1024:2048], ps1, Act.Exp, bias=mb_sb[:, kt : kt + 1]
                )
                atiles.append(at)
            if p > 0:
                finish_pair(p - 1, pav_prev)
            atn_prev = atiles

        for ot in range(CT):
            if ot not in pass1_done:
                emit_proj_pass1(ot)
        pav_last = [
            psum.tile(
                [128, 1024], F32, tag=f"pv{hi}", bufs=1,
                name=f"pav{2 * (NPAIR - 1) + hi}",
            )
            for hi in range(2)
        ]
        for kt in range(KT):
            emit_av_kt(NPAIR - 1, kt, atn_prev, pav_last)
        finish_pair(NPAIR - 1, pav_last)

        # warm-keeper: dependency-free matmuls keep the PE busy (HAM
        # clock-gate open) while the last normalization chain runs, so
        # proj pass 2 starts at full clock. Results are never read.
        for wi in range(12):
            wps = psum.tile([128, N], F32, tag="psS", bufs=2, name=f"warm{wi}")
            for qc in range(2):
                nc.tensor.matmul(
                    wps[:, qc * 512 : (qc + 1) * 512],
                    lhsT=wslice(wq_sb, 0, wi % CT),
                    rhs=xslice(xT_sb, N, wi % CT, qc * 512, (qc + 1) * 512),
                    start=True,
                    stop=True,
                )

        # ---------------- proj pass 2 (c-tiles 4..5 + partials) ----------
        for ot in range(CT):
            ps = psum.tile([128, N], F32, tag="psS", bufs=2, name=f"ps_p2_{ot}")
            for c in range(4, CT):
                for qc in range(2):
                    nc.tensor.matmul(
                        ps[:, qc * 512 : (qc + 1) * 512],
                        lhsT=wslice(pw_sb, ot, c),
                        rhs=aoT[c][:, qc * 512 : (qc + 1) * 512],
                        start=(c == 4),
                        stop=(c == CT - 1),
                    )
            osb = pool.tile([128, N], F32, tag="osb", bufs=2, name=f"o{ot}")
            nc.vector.tensor_add(out=osb, in0=ps, in1=osb1[ot])
            nc.sync.dma_start(out=out_d[ot * 128 : (ot + 1) * 128, :], in_=osb)

    nc.finalize()
    return nc


def kernel(x, mask, qkv_w, q_bias, v_bias, proj_w, proj_b, **_):
    x = np.asarray(x, np.float32)
    mask = np.asarray(mask)
    qkv_w = np.asarray(qkv_w, np.float32)
    q_bias = np.asarray(q_bias, np.float32)
    v_bias = np.asarray(v_bias, np.float32)
    proj_w = np.asarray(proj_w, np.float32)
    proj_b = np.asarray(proj_b, np.float32)

    nb = (~mask).sum(axis=1)
    KT = int(max(1, min(N // 128, -(-int(nb.max()) // 128))))
    NK = KT * 128

    wqT = np.ascontiguousarray(qkv_w[:C].T) * np.float32(SCALE)
    wkT = np.ascontiguousarray(qkv_w[C : 2 * C].T)
    wvT = np.ascontiguousarray(qkv_w[2 * C :].T)
    qkb = np.ascontiguousarray(
        (q_bias * SCALE).astype(np.float32).reshape(CT, 128).T
    )
    pb_eff = np.ascontiguousarray(
        (proj_b + proj_w @ v_bias).astype(np.float32).reshape(CT, 128).T
    )
    pwT = np.ascontiguousarray(proj_w.T)

    def grouped(wT):  # [C, C] -> [128, 6*768], cols grouped per out-block
        # host[part, g*768 + c*128 + j] = wT[c*128+part, g*128+j]
        return np.ascontiguousarray(
            wT.reshape(CT, 128, CT, 128).transpose(1, 2, 0, 3).reshape(128, CT * C)
        )

    def cmajor(mT):  # [C, W] -> [128, 6*W], plain c-major
        W = mT.shape[1]
        return np.ascontiguousarray(
            mT.reshape(CT, 128, W).transpose(1, 0, 2).reshape(128, CT * W)
        )

    wq_bf = grouped(wqT).astype(ml_dtypes.bfloat16)
    wk_bf = grouped(wkT).astype(ml_dtypes.bfloat16)
    wv_bf = cmajor(wvT).astype(ml_dtypes.bfloat16)
    pw_bf = grouped(pwT).astype(ml_dtypes.bfloat16)

    key = ("nc", KT)
    if key not in _CACHE:
        _CACHE[key] = _build_nc(KT)
    nc = _CACHE[key]
    _CACHE["nc"] = nc

    in_maps = []
    for b in range(B):
        xbT = np.ascontiguousarray(x[b].T).astype(ml_dtypes.bfloat16)
        idx = np.nonzero(~mask[b])[0]
        xk = np.zeros((C, NK), dtype=ml_dtypes.bfloat16)
        xk[:, : idx.size] = xbT[:, idx]
        mb = np.full(NK, MASK_NEG, np.float32)
        mb[: idx.size] = 0.0
        mb = np.ascontiguousarray(mb.reshape(KT, 128).T)
        in_maps.append(
            {
                "xT": cmajor(xbT),
                "xk": cmajor(xk),
                "wq": wq_bf,
                "wk": wk_bf,
                "wv": wv_bf,
                "pwT": pw_bf,
                "qkb": qkb,
                "mb": mb,
                "pb": pb_eff,
            }
        )

    _CACHE["last_in_maps"] = in_maps
    res = run_bass_kernel_spmd(nc, in_maps, list(range(B)))
    out = np.stack([res.results[b]["out"].T for b in range(B)], axis=0)
    return out.astype(np.float32)


if __name__ == "__main__":
    np.random.seed(0)
    x = np.random.randn(B, N, C).astype(np.float32)
    mask = np.random.randint(0, 2, (B, N)) > 0
    qkv_w = (np.random.randn(F3, C) * 0.02).astype(np.float32)
    q_bias = (np.random.randn(C) * 0.02).astype(np.float32)
    v_bias = (np.random.randn(C) * 0.02).astype(np.float32)
    proj_w = (np.random.randn(C, C) * 0.02).astype(np.float32)
    proj_b = (np.random.randn(C) * 0.02).astype(np.float32)
    out = kernel(x, mask, qkv_w, q_bias, v_bias, proj_w, proj_b)
    print(out.shape, out.dtype)


# revision 19
# speedup vs baseline: 1.0404x; 1.0404x over previous
"""Trainium2 Bass kernel for AttentionWithCAE.

Reference computation (B=8, N=1024, C=768, H=12, hd=64):
    qkv  = x @ qkv_w.T + concat(q_bias, 0, v_bias)
    q,k,v per head; attn = softmax(mask(q*scale @ k.T)); out = attn @ v
    final = out @ proj_w.T + proj_b

Sharding: pure data parallel -- batch b on core b, weights replicated,
no collectives.

Key optimization vs the v1 kernel: masked keys (~50% of the 1024) are
removed on the HOST (exact algebra: their exp() is 0, they contribute
nothing). Each batch's unmasked keys are gathered into a compact
[C, NK] tensor (NK = KT*128, KT = ceil(max_b n_unmasked / 128), 5 for
the reference data), cutting the scores matmuls, exp() activations,
attn@v matmuls and the k/v projections by ~40%.

Device-side layout (per core, all zero-transpose):
  - xT [C, N] bf16 full tokens (q path); xk [C, NK] bf16 gathered keys
    (k/v path); wq (pre-scaled by SCALE) / wk / wv [C, 768] bf16;
    pwT [C, C] bf16.
  - qT[p] [128, N]: q features of heads 2p (rows 0:64) / 2p+1 (64:128);
    kT[p] [128, NK] likewise: exactly the lhsT/rhs layout the scores
    matmul needs (contraction over head_dim; two concurrent K=64
    matmuls on PE row halves).
  - scores computed transposed [k, q]; the key-dependent pad-mask bias
    is a per-partition bias folded into the Exp activation.
  - v emitted token-major into v65 tiles [128, 12*65]: per head 64
    v-columns plus a baked ones column, so each attn@v matmul also
    yields the softmax denominators (PSUM row 64).
  - denominators -> reciprocal (direct from PSUM row) -> DRAM-bounce
    partition-broadcast -> normalize into aoT [C-major, N] which feeds
    proj; v_bias folds into an effective proj bias on the host.

Schedule: the steady state is paced by the Exp ACTs (2 per (pair, kt)
on ScalarE). Per kt iteration the PE runs 4 scores matmuls (row-packed
pairs) + 4 attn@v matmuls of the previous pair; the next pair's q/k
projection tiles are emitted inside the kt loop (kt==2/kt==4) where the
ScalarE backlog hides them. PSUM: 4 banks scores (double-buffered
[128,1024], rotation shared with q/k-proj and final proj) + 4 banks AV.
"""

import sys

sys.path.insert(0, "/opt/trn_rl_repo")

from contextlib import ExitStack

import numpy as np
import ml_dtypes

import concourse.bass as bass
import concourse.bacc as bacc
import concourse.mybir as mybir
from concourse import tile
from concourse.bass_utils import run_bass_kernel_spmd
from concourse.masks import make_identity

B, N, C = 8, 1024, 768
H, HD = 12, 64
F3 = 3 * C
SCALE = HD ** -0.5
F32 = mybir.dt.float32
BF16 = mybir.dt.bfloat16
Act = mybir.ActivationFunctionType

MASK_NEG = -30000.0

CT = C // 128  # 6 contraction tiles
NPAIR = H // 2  # 6 head pairs

_CACHE = {}


def _build_nc(KT):
    NK = KT * 128
    nc = bacc.Bacc(None, target_bir_lowering=False)

    # All matrix inputs arrive in a host-transposed "mega-tile" layout
    # [128, CT*cols]: partition p, column c*cols+j holds element
    # [c*128+p, j] of the logical [C, cols] matrix. This makes every DMA
    # per-partition contiguous (1.5-4KB packets instead of the
    # sub-2KB strided lines that made the v1 input load DMA-overhead
    # bound). wq/wk/pw additionally group columns by use order (see
    # kernel()) so pair-0 weight chunks can load first.
    xT_d = nc.declare_dram_parameter("xT", [128, CT * N], BF16, isOutput=False)
    xk_d = nc.declare_dram_parameter("xk", [128, CT * NK], BF16, isOutput=False)
    wq_d = nc.declare_dram_parameter("wq", [128, CT * C], BF16, isOutput=False)
    wk_d = nc.declare_dram_parameter("wk", [128, CT * C], BF16, isOutput=False)
    wv_d = nc.declare_dram_parameter("wv", [128, CT * C], BF16, isOutput=False)
    pw_d = nc.declare_dram_parameter("pwT", [128, CT * C], BF16, isOutput=False)
    # biases arrive host-transposed [128, tiles] so the DMA is contiguous
    qkb_d = nc.declare_dram_parameter("qkb", [128, CT], F32, isOutput=False)
    mb_d = nc.declare_dram_parameter("mb", [128, KT], F32, isOutput=False)
    pb_d = nc.declare_dram_parameter("pb", [128, CT], F32, isOutput=False)
    out_d = nc.declare_dram_parameter("out", [C, N], F32, isOutput=True)

    r_d = nc.dram_tensor("r_scratch", [H, N], F32)

    with ExitStack() as ctx:
        tc = ctx.enter_context(tile.TileContext(nc))
        pool = ctx.enter_context(tc.tile_pool(name="main", bufs=1))
        psum = ctx.enter_context(tc.tile_pool(name="psum", bufs=1, space="PSUM"))

        qb_sb = pool.tile([128, CT], F32)
        mb_sb = pool.tile([128, KT], F32)
        pb_sb = pool.tile([128, CT], F32)

        # Mega-tiles in SBUF. Slice helpers below recover the per-c-tile
        # views the matmuls need.
        wq_sb = pool.tile([128, CT * C], BF16, name="wq_sb")
        wk_sb = pool.tile([128, CT * C], BF16, name="wk_sb")
        wv_sb = pool.tile([128, CT * C], BF16, name="wv_sb")
        pw_sb = pool.tile([128, CT * C], BF16, name="pw_sb")
        xT_sb = pool.tile([128, CT * N], BF16, name="xT_sb")
        xk_sb = pool.tile([128, CT * NK], BF16, name="xk_sb")

        def chunk_dma(sb, dr, n_chunks):
            w = sb.shape[1]
            step = w // n_chunks
            for i in range(n_chunks):
                nc.sync.dma_start(
                    out=sb[:, i * step : (i + 1) * step],
                    in_=dr[:, i * step : (i + 1) * step],
                )

        # Load order = first use: pair-0 q/k weight chunks + xT + xk first,
        # then the remaining pairs' weight chunks, v weights, proj weights.
        # wq/wk layout groups 768 columns per pair (6 c-slices of 128).
        nc.sync.dma_start(out=wq_sb[:, 0:C], in_=wq_d[:, 0:C])
        chunk_dma(xT_sb, xT_d, 3)
        nc.sync.dma_start(out=wk_sb[:, 0:C], in_=wk_d[:, 0:C])
        nc.sync.dma_start(out=qb_sb, in_=qkb_d[:, :])
        chunk_dma(xk_sb, xk_d, 2)
        nc.sync.dma_start(out=mb_sb, in_=mb_d[:, :])
        chunk_dma(wv_sb, wv_d, 2)
        for p in range(1, NPAIR):
            nc.sync.dma_start(
                out=wq_sb[:, p * C : (p + 1) * C], in_=wq_d[:, p * C : (p + 1) * C]
            )
            nc.sync.dma_start(
                out=wk_sb[:, p * C : (p + 1) * C], in_=wk_d[:, p * C : (p + 1) * C]
            )
        chunk_dma(pw_sb, pw_d, 2)
        nc.sync.dma_start(out=pb_sb, in_=pb_d[:, :])

        # wq/wk/pw column j of logical tile (group, c) = group*C + c*128 + j
        def wslice(sb, group, c):
            return sb[:, group * C + c * 128 : group * C + (c + 1) * 128]

        # xT/xk column t of c-tile c = c*width + t
        def xslice(sb, width, c, lo, hi):
            return sb[:, c * width + lo : c * width + hi]

        qT = [
            pool.tile([128, N], BF16, tag="qT", bufs=NPAIR, name=f"qT{p}")
            for p in range(NPAIR)
        ]
        kT = [
            pool.tile([128, NK], BF16, tag="kT", bufs=NPAIR, name=f"kT{p}")
            for p in range(NPAIR)
        ]
        v65 = [
            pool.tile([128, H * 65], BF16, tag="v65", bufs=KT, name=f"v65_{i}")
            for i in range(KT)
        ]
        aoT = [
            pool.tile([128, N], BF16, tag="aoT", bufs=CT, name=f"aoT{i}")
            for i in range(CT)
        ]

        qps = {}

        def emit_q_half(p, half):
            # q-projection emitted in two 3-c-tile halves so no single PE
            # burst outruns the ScalarE exp backlog
            if half == 0:
                qps[p] = psum.tile([128, N], F32, tag="psS", bufs=2, name=f"ps_q{p}")
            ps = qps[p]
            for c in range(3 * half, 3 * half + 3):
                for qc in range(2):
                    nc.tensor.matmul(
                        ps[:, qc * 512 : (qc + 1) * 512],
                        lhsT=wslice(wq_sb, p, c),
                        rhs=xslice(xT_sb, N, c, qc * 512, (qc + 1) * 512),
                        start=(c == 0),
                        stop=(c == CT - 1),
                    )
            if half == 1:
                nc.vector.tensor_scalar_add(
                    out=qT[p], in0=ps, scalar1=qb_sb[:, p : p + 1]
                )

        kps = {}

        def emit_k_half(p, half):
            if half == 0:
                kps[p] = psum.tile([128, N], F32, tag="psS", bufs=2, name=f"ps_k{p}")
            ps = kps[p]
            chunks = [(0, min(NK, 512))]
            if NK > 512:
                chunks.append((512, NK))
            for c in range(3 * half, 3 * half + 3):
                for lo, hi in chunks:
                    nc.tensor.matmul(
                        ps[:, lo:hi],
                        lhsT=wslice(wk_sb, p, c),
                        rhs=xslice(xk_sb, NK, c, lo, hi),
                        start=(c == 0),
                        stop=(c == CT - 1),
                    )
            if half == 1:
                nc.vector.tensor_copy(out=kT[p], in_=ps[:, 0:NK])

        def emit_q_tile(p):
            emit_q_half(p, 0)
            emit_q_half(p, 1)

        def emit_k_tile(p):
            emit_k_half(p, 0)
            emit_k_half(p, 1)

        def emit_v_tile(kt):
            vps = psum.tile(
                [128, 768], F32, tag=f"pv{kt % 2}", bufs=1, name=f"ps_v{kt}"
            )
            for c in range(CT):
                nc.tensor.matmul(
                    vps[:, 0:512],
                    lhsT=xslice(xk_sb, NK, c, kt * 128, (kt + 1) * 128),
                    rhs=xslice(wv_sb, C, c, 0, 512),
                    start=(c == 0),
                    stop=(c == CT - 1),
                )
                nc.tensor.matmul(
                    vps[:, 512:768],
                    lhsT=xslice(xk_sb, NK, c, kt * 128, (kt + 1) * 128),
                    rhs=xslice(wv_sb, C, c, 512, 768),
                    start=(c == 0),
                    stop=(c == CT - 1),
                )
            v3 = v65[kt].rearrange("p (h j) -> p h j", j=65)
            nc.vector.tensor_copy(
                out=v3[:, :, 0:64], in_=vps.rearrange("p (h j) -> p h j", j=64)
            )
            nc.vector.memset(v3[:, :, 64:65], 1.0)

        def emit_av_kt(p, kt, atiles, pav):
            for hi in range(2):
                h = 2 * p + hi
                for qc in range(2):
                    nc.tensor.matmul(
                        pav[hi][0:65, qc * 512 : (qc + 1) * 512],
                        lhsT=v65[kt][:, h * 65 : (h + 1) * 65],
                        rhs=atiles[kt][
                            :, hi * 1024 + qc * 512 : hi * 1024 + (qc + 1) * 512
                        ],
                        start=(kt == 0),
                        stop=(kt == KT - 1),
                    )

        def finish_pair(p, pav, last=False):
            # Fast [65, 512] copies evict the unnormalized AV plus the
            # denominator row (PSUM row 64) to SBUF right away, releasing the
            # PSUM banks for the next pair's AV. The slow normalization chain
            # (reciprocal -> DRAM-bounce broadcast -> mul) then runs entirely
            # from SBUF. NOTE: reciprocal_approx_fast directly on a PSUM
            # source returns garbage on HW (sim accepts it).
            for hi in range(2):
                h = 2 * p + hi
                srow = pool.tile([1, N], F32, tag="srow", bufs=2, name=f"s{h}")
                if last:
                    # nothing needs these PSUM banks afterwards: normalize
                    # straight from PSUM, skipping the eviction copy
                    un = pav[hi][0:64, :]
                else:
                    un = pool.tile(
                        [64, N], F32, tag=f"un{hi}", bufs=2, name=f"un{h}"
                    )
                    nc.vector.tensor_copy(out=un, in_=pav[hi][0:64, :])
                # NOTE: reciprocal_approx_fast needs a partition-0 SBUF
                # source (PSUM or partition-offset sources return garbage
                # on HW; sim accepts both) — hence the srow bounce.
                nc.vector.tensor_copy(out=srow, in_=pav[hi][64:65, :])
                r_row = pool.tile([1, N], F32, tag="rrow", bufs=2, name=f"r{h}")
                nc.vector.reciprocal_approx_fast(out=r_row, in_=srow)
                nc.sync.dma_start(out=r_d[h : h + 1, :], in_=r_row)
                r2 = pool.tile([64, N], F32, tag="r2", bufs=4, name=f"r2_{h}")
                nc.sync.dma_start(out=r2, in_=r_d[h : h + 1, :].to_broadcast([64, N]))
                # normalization muls split across the otherwise-idle GPSIMD
                # (all-SBUF operands) and DVE so the two head-chains overlap
                eng = nc.vector if last else (nc.gpsimd if hi == 0 else nc.vector)
                eng.tensor_mul(
                    out=aoT[p][hi * 64 : (hi + 1) * 64, :],
                    in0=un,
                    in1=r2,
                )

        # proj pass 1: c-tiles 0..3 (pairs 0..3, ready once finish_pair(3)
        # ran) accumulated into bf16 partials with the output bias; pass 2
        # adds c-tiles 4..5 after the last pair finishes. Pass 1 is emitted
        # inside pair 5's kt loop where the PE has slack under the exp pace.
        osb1 = [
            pool.tile([128, N], BF16, tag="osb1", bufs=CT, name=f"op1_{i}")
            for i in range(CT)
        ]

        def emit_proj_pass1(ot):
            ps = psum.tile([128, N], F32, tag="psS", bufs=2, name=f"ps_p1_{ot}")
            for c in range(4):
                for qc in range(2):
                    nc.tensor.matmul(
                        ps[:, qc * 512 : (qc + 1) * 512],
                        lhsT=wslice(pw_sb, ot, c),
                        rhs=aoT[c][:, qc * 512 : (qc + 1) * 512],
                        start=(c == 0),
                        stop=(c == 3),
                    )
            nc.scalar.activation(
                osb1[ot], ps, Act.Identity, bias=pb_sb[:, ot : ot + 1]
            )

        eye_sb = pool.tile([128, 128], BF16, name="eye_sb")
        make_identity(nc, eye_sb)

        emit_q_tile(0)
        emit_k_tile(0)
        pass1_done = set()
        pav_prev = None
        atn_prev = None
        for p in range(NPAIR):
            if p > 0:
                pav_prev = [
                    psum.tile(
                        [128, 1024], F32, tag=f"pv{hi}", bufs=1,
                        name=f"pav{2 * (p - 1) + hi}",
                    )
                    for hi in range(2)
                ]
            atiles = []
            for kt in range(KT):
                # PE filler work first (so later scores/exps never wait
                # behind it via psum-slot WAR in the same engine stream)
                if p < NPAIR - 1:
                    # self-contained inserts (alloc+matmuls+evict) only: a
                    # psum tile held across kt iterations would deadlock the
                    # 2-deep psS rotation
                    nxt = p + 1
                    if kt == min(1, KT - 1):
                        emit_q_tile(nxt)
                    if kt == min(3, KT - 1):
                        emit_k_tile(nxt)
                else:
                    if kt < 5 and kt < CT:
                        emit_proj_pass1(kt)
                        pass1_done.add(kt)
                ps0 = psum.tile(
                    [128, N], F32, tag="psS", bufs=2, name=f"ps_s{2 * p}_{kt}"
                )
                ps1 = psum.tile(
                    [128, N], F32, tag="psS", bufs=2, name=f"ps_s{2 * p + 1}_{kt}"
                )
                def score_mms(hi, ps):
                    for qc in range(2):
                        # row-packed: even head rows 0-63, odd head 64-127
                        nc.tensor.matmul(
                            ps[:, qc * 512 : (qc + 1) * 512],
                            lhsT=kT[p][
                                hi * 64 : hi * 64 + 64, kt * 128 : (kt + 1) * 128
                            ],
                            rhs=qT[p][hi * 64 : hi * 64 + 64, qc * 512 : (qc + 1) * 512],
                            start=True,
                            stop=True,
                        )

                def av_mms(hi):
                    h = 2 * (p - 1) + hi
                    for qc in range(2):
                        nc.tensor.matmul(
                            pav_prev[hi][0:65, qc * 512 : (qc + 1) * 512],
                            lhsT=v65[kt][:, h * 65 : (h + 1) * 65],
                            rhs=atn_prev[kt][
                                :, hi * 1024 + qc * 512 : hi * 1024 + (qc + 1) * 512
                            ],
                            start=(kt == 0),
                            stop=(kt == KT - 1),
                        )

                # AV matmuls interleave between the two heads' score matmuls
                # so the PE has work while the DVE evicts an inserted q/k tile
                score_mms(0, ps0)
                if p > 0:
                    av_mms(0)
                score_mms(1, ps1)
                if p > 0:
                    av_mms(1)
                if p == 0:
                    emit_v_tile(kt)
                at = pool.tile(
                    [128, 2048], BF16, tag="attn", bufs=2 * KT, name=f"at{p}_{kt}"
                )
                nc.scalar.activation(
                    at[:, 0:1024], ps0, Act.Exp, bias=mb_sb[:, kt : kt + 1]
                )
                nc.scalar.activation(
                    at[:, 1024:2048], ps1, Act.Exp, bias=mb_sb[:, kt : kt + 1]
                )
                atiles.append(at)
            if p > 0:
                finish_pair(p - 1, pav_prev)
            atn_prev = atiles

        for ot in range(CT):
            if ot not in pass1_done:
                emit_proj_pass1(ot)
        pav_last = [
            psum.tile(
                [128, 1024], F32, tag=f"pv{hi}", bufs=1,
                name=f"pav{2 * (NPAIR - 1) + hi}",
            )
            for hi in range(2)
        ]
        for kt in range(KT):
            emit_av_kt(NPAIR - 1, kt, atn_prev, pav_last)
        finish_pair(NPAIR - 1, pav_last, last=True)

        # warm-keeper: dependency-free matmuls keep the PE busy (HAM
        # clock-gate open) while the last normalization chain runs, so
        # proj pass 2 starts at full clock. Results are never read.
        for wi in range(12):
            wps = psum.tile([128, N], F32, tag="psS", bufs=2, name=f"warm{wi}")
            for qc in range(2):
                nc.tensor.matmul(
                    wps[:, qc * 512 : (qc + 1) * 512],
                    lhsT=wslice(wq_sb, 0, wi % CT),
                    rhs=xslice(xT_sb, N, wi % CT, qc * 512, (qc + 1) * 512),
                    start=True,
                    stop=True,
                )

        # ---------------- proj pass 2 (c-tiles 4..5 + partials) ----------
        # the bf16 pass-1 partial re-enters the accumulation as an
        # identity matmul (cheaper than a serial DVE add per tile)
        for ot in range(CT):
            ps = psum.tile([128, N], F32, tag="psS", bufs=2, name=f"ps_p2_{ot}")
            for c in range(4, CT):
                for qc in range(2):
                    nc.tensor.matmul(
                        ps[:, qc * 512 : (qc + 1) * 512],
                        lhsT=wslice(pw_sb, ot, c),
                        rhs=aoT[c][:, qc * 512 : (qc + 1) * 512],
                        start=(c == 4),
                        stop=False,
                    )
            for qc in range(2):
                nc.tensor.matmul(
                    ps[:, qc * 512 : (qc + 1) * 512],
                    lhsT=eye_sb,
                    rhs=osb1[ot][:, qc * 512 : (qc + 1) * 512],
                    start=False,
                    stop=True,
                )
            osb = pool.tile([128, N], F32, tag="osb", bufs=2, name=f"o{ot}")
            nc.scalar.activation(osb, ps, Act.Copy)
            nc.sync.dma_start(out=out_d[ot * 128 : (ot + 1) * 128, :], in_=osb)

    nc.finalize()
    return nc


def kernel(x, mask, qkv_w, q_bias, v_bias, proj_w, proj_b, **_):
    x = np.asarray(x, np.float32)
    mask = np.asarray(mask)
    qkv_w = np.asarray(qkv_w, np.float32)
    q_bias = np.asarray(q_bias, np.float32)
    v_bias = np.asarray(v_bias, np.float32)
    proj_w = np.asarray(proj_w, np.float32)
    proj_b = np.asarray(proj_b, np.float32)

    nb = (~mask).sum(axis=1)
    KT = int(max(1, min(N // 128, -(-int(nb.max()) // 128))))
    NK = KT * 128

    wqT = np.ascontiguousarray(qkv_w[:C].T) * np.float32(SCALE)
    wkT = np.ascontiguousarray(qkv_w[C : 2 * C].T)
    wvT = np.ascontiguousarray(qkv_w[2 * C :].T)
    qkb = np.ascontiguousarray(
        (q_bias * SCALE).astype(np.float32).reshape(CT, 128).T
    )
    pb_eff = np.ascontiguousarray(
        (proj_b + proj_w @ v_bias).astype(np.float32).reshape(CT, 128).T
    )
    pwT = np.ascontiguousarray(proj_w.T)

    def grouped(wT):  # [C, C] -> [128, 6*768], cols grouped per out-block
        # host[part, g*768 + c*128 + j] = wT[c*128+part, g*128+j]
        return np.ascontiguousarray(
            wT.reshape(CT, 128, CT, 128).transpose(1, 2, 0, 3).reshape(128, CT * C)
        )

    def cmajor(mT):  # [C, W] -> [128, 6*W], plain c-major
        W = mT.shape[1]
        return np.ascontiguousarray(
            mT.reshape(CT, 128, W).transpose(1, 0, 2).reshape(128, CT * W)
        )

    wq_bf = grouped(wqT).astype(ml_dtypes.bfloat16)
    wk_bf = grouped(wkT).astype(ml_dtypes.bfloat16)
    wv_bf = cmajor(wvT).astype(ml_dtypes.bfloat16)
    pw_bf = grouped(pwT).astype(ml_dtypes.bfloat16)

    key = ("nc", KT)
    if key not in _CACHE:
        _CACHE[key] = _build_nc(KT)
    nc = _CACHE[key]
    _CACHE["nc"] = nc

    in_maps = []
    for b in range(B):
        xbT = np.ascontiguousarray(x[b].T).astype(ml_dtypes.bfloat16)
        idx = np.nonzero(~mask[b])[0]
        xk = np.zeros((C, NK), dtype=ml_dtypes.bfloat16)
        xk[:, : idx.size] = xbT[:, idx]
        mb = np.full(NK, MASK_NEG, np.float32)
        mb[: idx.size] = 0.0
        mb = np.ascontiguousarray(mb.reshape(KT, 128).T)
        in_maps.append(
            {
                "xT": cmajor(xbT),
                "xk": cmajor(xk),
                "wq": wq_bf,
                "wk": wk_bf,
                "wv": wv_bf,
                "pwT": pw_bf,
                "qkb": qkb,
                "mb": mb,
                "pb": pb_eff,
            }
        )

    _CACHE["last_in_maps"] = in_maps
    res = run_bass_kernel_spmd(nc, in_maps, list(range(B)))
    out = np.stack([res.results[b]["out"].T for b in range(B)], axis=0)
    return out.astype(np.float32)


if __name__ == "__main__":
    np.random.seed(0)
    x = np.random.randn(B, N, C).astype(np.float32)
    mask = np.random.randint(0, 2, (B, N)) > 0
    qkv_w = (np.random.randn(F3, C) * 0.02).astype(np.float32)
    q_bias = (np.random.randn(C) * 0.02).astype(np.float32)
    v_bias = (np.random.randn(C) * 0.02).astype(np.float32)
    proj_w = (np.random.randn(C, C) * 0.02).astype(np.float32)
    proj_b = (np.random.randn(C) * 0.02).astype(np.float32)
    out = kernel(x, mask, qkv_w, q_bias, v_bias, proj_w, proj_b)
    print(out.shape, out.dtype)
